# revision 1
# baseline (speedup 1.0000x reference)
"""GCNConv mean-aggregation kernel for 8 Trainium2 NeuronCores.

Reference computation:
    msgs   = x[src]                       # [E, D] gather
    summed = segment_sum(msgs, dst, N)    # [N, D]
    deg    = segment_sum(ones, dst, N)    # [N]
    h      = summed / max(deg, 1)
    out    = h @ W.T + b                  # [N, D_OUT]

Strategy (no collectives needed):
  - Shard edges by contiguous dst ranges: core c owns nodes
    [c*6272, (c+1)*6272).  Each core fully reduces its own node range.
  - Per core the edge stream is grouped into 64-node dst windows.  For
    each 128-edge subtile we gather x[src] rows from HBM with big
    dma_gather calls (512B rows: 64 feats + 1.0 weight col + pad)
    rotated over the 4 SWDGE queues (single_packet=False; the default
    single-packet mode wedges the SDMA engine beyond ~64 descs/lane),
    build a [128e, 64n] one-hot from dst via a DVE is_equal against an
    iota, and accumulate  onehot.T @ msgs  into a [64, 65] PSUM tile
    (features + degree in one matmul chain).
  - Normalize by max(deg,1) with per-partition scalars, transpose h via
    the PE identity trick, apply W (as lhsT = W.T) and bias, and write
    out.T slices ([64, 6272] per core).  Host reassembles/transposes.
  - dma_gather indices are int16, so x is staged into two gather tables
    (src < 32767 and src >= 32767), each with a zero row at index 0
    used by padding edges (contributes 0 to sums and degree).
"""

import sys

sys.path.insert(0, "/opt/trn_rl_repo")

import numpy as np

import concourse.bacc as bacc
import concourse.mybir as mybir
import concourse.tile as tile
from concourse.bass_utils import run_bass_kernel_spmd

N_NODES = 50000
N_EDGES = 800000
D = 64
N_CORES = 8
NPC = 6272          # nodes per core (= 98 windows of 64 = 49 tiles of 128)
WIN = 64            # dst-window width per PSUM accumulation group
N_WIN = NPC // WIN  # 98
SPLIT = 32767       # src < SPLIT -> lo table, else hi table
ROW = 128           # gather row: 64 feats + weight + zero pad (512 B)
CHUNK = 16          # subtiles (of 128 edges) per dma_gather call
NQ = 4              # SWDGE queues for parallel gather descriptor work

F32 = mybir.dt.float32
I16 = mybir.dt.int16

# Results of the most recent run (for test harness inspection).
LAST = {}


def _prep(x, src, dst):
    """Host-side sharding: build gather tables, per-core padded edge
    streams (int16 gather idx + f32 dst-rel), and per-window subtile
    budgets (shared across cores; SPMD program structure)."""
    x = np.asarray(x, dtype=np.float32)
    src = np.asarray(src, dtype=np.int64)
    dst = np.asarray(dst, dtype=np.int64)

    n_lo = SPLIT
    n_hi = N_NODES - SPLIT
    xlo = np.zeros((n_lo + 1, ROW), dtype=np.float32)
    xlo[1:, :D] = x[:SPLIT]
    xlo[1:, D] = 1.0
    xhi = np.zeros((n_hi + 1, ROW), dtype=np.float32)
    xhi[1:, :D] = x[SPLIT:]
    xhi[1:, D] = 1.0

    gw = (dst // WIN).astype(np.int64)
    cls = (src >= SPLIT).astype(np.int64)
    key = gw * 2 + cls
    order = np.argsort(key, kind="stable")
    src_s, dst_s = src[order], dst[order]

    n_groups = (N_CORES * N_WIN) * 2
    counts = np.bincount(key[order], minlength=n_groups)
    starts = np.zeros(n_groups + 1, dtype=np.int64)
    np.cumsum(counts, out=starts[1:])

    cnt = counts.reshape(N_CORES, N_WIN, 2)
    kA = np.maximum(1, -(-cnt[:, :, 0].max(axis=0) // 128))  # [N_WIN]
    kB = -(-cnt[:, :, 1].max(axis=0) // 128)                  # [N_WIN]
    SA = int(kA.sum())
    SB = int(kB.sum())

    idx_lo = (src_s + 1).astype(np.int16)
    idx_hi = (src_s - SPLIT + 1).astype(np.int16)

    offA = np.zeros(N_WIN + 1, dtype=np.int64)
    np.cumsum(kA, out=offA[1:])
    offB = np.zeros(N_WIN + 1, dtype=np.int64)
    np.cumsum(kB, out=offB[1:])

    per_core = []
    for c in range(N_CORES):
        iA = np.zeros(SA * 128, dtype=np.int16)
        dA = np.zeros(SA * 128, dtype=np.float32)
        iB = np.zeros(max(SB, 1) * 128, dtype=np.int16)
        dB = np.zeros(max(SB, 1) * 128, dtype=np.float32)
        for w in range(N_WIN):
            g = (c * N_WIN + w) * 2
            base = (c * N_WIN + w) * WIN
            s0, s1 = starts[g], starts[g + 1]
            p0 = int(offA[w]) * 128
            iA[p0 : p0 + (s1 - s0)] = idx_lo[s0:s1]
            dA[p0 : p0 + (s1 - s0)] = (dst_s[s0:s1] - base).astype(np.float32)
            s0, s1 = starts[g + 1], starts[g + 2]
            p0 = int(offB[w]) * 128
            iB[p0 : p0 + (s1 - s0)] = idx_hi[s0:s1]
            dB[p0 : p0 + (s1 - s0)] = (dst_s[s0:s1] - base).astype(np.float32)
        per_core.append((iA, dA, iB, dB))

    return xlo, xhi, kA, kB, SA, SB, offA, offB, per_core


def _wrap_idx(idx_flat):
    """int16 stream -> dma_gather layout [128, n/16]: value i at
    [i % 16, i // 16], replicated across the 8 groups of 16 partitions."""
    a = idx_flat.reshape(-1, 16).T
    return np.tile(a, (8, 1)).copy()


def _wrap_dst(d_flat):
    """f32 stream -> [128, S]: subtile s lane e at [e, s]."""
    return np.ascontiguousarray(d_flat.reshape(-1, 128).T)


def _build_program(kA, kB, SA, SB, offA, offB):
    nc = bacc.Bacc(
        "TRN2", target_bir_lowering=False, debug=False, num_swdge_queues=NQ
    )

    t_xlo = nc.dram_tensor("xlo", [SPLIT + 1, ROW], F32, kind="ExternalInput")
    t_xhi = nc.dram_tensor(
        "xhi", [N_NODES - SPLIT + 1, ROW], F32, kind="ExternalInput"
    )
    t_wt = nc.dram_tensor("wt", [D, D], F32, kind="ExternalInput")
    t_b = nc.dram_tensor("bias", [D, 1], F32, kind="ExternalInput")
    t_ia = nc.dram_tensor("idxa", [128, SA * 8], I16, kind="ExternalInput")
    t_da = nc.dram_tensor("dsta", [128, SA], F32, kind="ExternalInput")
    SBp = max(SB, 1)
    t_ib = nc.dram_tensor("idxb", [128, SBp * 8], I16, kind="ExternalInput")
    t_db = nc.dram_tensor("dstb", [128, SBp], F32, kind="ExternalInput")
    t_iota = nc.dram_tensor("iota", [128, CHUNK * WIN], F32, kind="ExternalInput")
    t_id = nc.dram_tensor("ident", [D, D], F32, kind="ExternalInput")
    t_out = nc.dram_tensor("out", [D, NPC], F32, kind="ExternalOutput")

    callsA = [(p, min(CHUNK, SA - p)) for p in range(0, SA, CHUNK)]
    callsB = [(p, min(CHUNK, SB - p)) for p in range(0, SB, CHUNK)]

    with tile.TileContext(nc) as tc:
        with (
            tc.tile_pool(name="const", bufs=1) as cpool,
            tc.tile_pool(name="idx", bufs=1) as ipool,
            tc.tile_pool(name="msgsa", bufs=4) as mpa,
            tc.tile_pool(name="msgsb", bufs=3) as mpb,
            tc.tile_pool(name="oha", bufs=4) as opa,
            tc.tile_pool(name="ohb", bufs=3) as opb,
            tc.tile_pool(name="norm", bufs=4) as npool,
            tc.tile_pool(name="hpo", bufs=2) as hpool,
            tc.tile_pool(name="psacc", bufs=4, space="PSUM") as ps_acc,
            tc.tile_pool(name="pstr", bufs=2, space="PSUM") as ps_tr,
            tc.tile_pool(name="psz", bufs=2, space="PSUM") as ps_z,
        ):
            # ---- constants (iota / identity supplied from host) ----
            ident = cpool.tile([D, D], F32)
            nc.sync.dma_start(out=ident[:], in_=t_id[:])
            wt_sb = cpool.tile([D, D], F32)
            nc.sync.dma_start(out=wt_sb[:], in_=t_wt[:])
            b_sb = cpool.tile([D, 1], F32)
            nc.sync.dma_start(out=b_sb[:], in_=t_b[:])
            iota_f = cpool.tile([128, CHUNK * WIN], F32)
            nc.sync.dma_start(out=iota_f[:], in_=t_iota[:])

            ia_sb = ipool.tile([128, SA * 8], I16)
            nc.sync.dma_start(out=ia_sb[:], in_=t_ia[:])
            da_sb = ipool.tile([128, SA], F32)
            nc.sync.dma_start(out=da_sb[:], in_=t_da[:])
            ib_sb = ipool.tile([128, SBp * 8], I16)
            nc.sync.dma_start(out=ib_sb[:], in_=t_ib[:])
            db_sb = ipool.tile([128, SBp], F32)
            nc.sync.dma_start(out=db_sb[:], in_=t_db[:])

            out_sb = cpool.tile([D, NPC], F32)

            chunk_tiles = {0: [], 1: []}
            call_no = [0]

            def emit_chunk(st, k):
                if st == 0:
                    pos, nsub = callsA[k]
                    mp, op, tsrc, isb, dsb = mpa, opa, t_xlo, ia_sb, da_sb
                else:
                    pos, nsub = callsB[k]
                    mp, op, tsrc, isb, dsb = mpb, opb, t_xhi, ib_sb, db_sb
                msgs = mp.tile([128, CHUNK, ROW], F32)
                nidx = nsub * 128
                # single_packet=False: one packet per descriptor. The default
                # coalesces the whole call into one SDMA packet, which wedges
                # the engine beyond ~64 descriptors/lane (num_idxs >~ 1000).
                # Rotating queue_num spreads descriptor generation + ring
                # drain over the 4 SWDGE queues (~2x measured).
                nc.gpsimd.dma_gather(
                    msgs[:, :nsub, :],
                    tsrc[:],
                    isb[:, pos * 8 : pos * 8 + nsub * 8],
                    nidx,
                    nidx,
                    ROW,
                    single_packet=False,
                    queue_num=call_no[0] % NQ,
                )
                call_no[0] += 1
                oh = op.tile([128, CHUNK * WIN], F32)
                dst_b = (
                    dsb[:, pos : pos + nsub]
                    .unsqueeze(2)
                    .to_broadcast([128, nsub, WIN])
                )
                nc.vector.tensor_tensor(
                    out=oh[:, : nsub * WIN].rearrange("p (s w) -> p s w", w=WIN),
                    in0=iota_f[:, : nsub * WIN].rearrange(
                        "p (s w) -> p s w", w=WIN
                    ),
                    in1=dst_b,
                    op=mybir.AluOpType.is_equal,
                )
                chunk_tiles[st].append((msgs, oh))

            cursor = [0, 0]

            def tiles_for(st, s):
                k = s // CHUNK
                while cursor[st] <= k:
                    emit_chunk(st, cursor[st])
                    cursor[st] += 1
                msgs, oh = chunk_tiles[st][k]
                return msgs, oh, s % CHUNK

            pst = None
            for w in range(N_WIN):
                subs = [(0, int(offA[w]) + j) for j in range(int(kA[w]))]
                subs += [(1, int(offB[w]) + j) for j in range(int(kB[w]))]
                ps = ps_acc.tile([WIN, D + 1], F32)
                for j, (st, s) in enumerate(subs):
                    msgs, oh, col = tiles_for(st, s)
                    nc.tensor.matmul(
                        out=ps[:],
                        lhsT=oh[:, col * WIN : (col + 1) * WIN],
                        rhs=msgs[:, col, : D + 1],
                        start=(j == 0),
                        stop=(j == len(subs) - 1),
                    )
                deg = npool.tile([WIN, 1], F32)
                nc.vector.tensor_scalar_max(deg[:], ps[:, D : D + 1], 1.0)
                rec = npool.tile([WIN, 1], F32)
                nc.vector.reciprocal(rec[:], deg[:])
                h_w = npool.tile([WIN, D], F32)
                nc.vector.tensor_scalar_mul(h_w[:], ps[:, :D], rec[:])
                half = w % 2
                if half == 0:
                    pst = ps_tr.tile([D, 128], F32)
                nc.tensor.transpose(
                    out=pst[:, half * WIN : half * WIN + WIN],
                    in_=h_w[:],
                    identity=ident[:],
                )
                if half == 1:
                    ht = hpool.tile([D, 128], F32)
                    nc.vector.tensor_copy(out=ht[:], in_=pst[:])
                    z = ps_z.tile([D, 128], F32)
                    nc.tensor.matmul(
                        out=z[:], lhsT=wt_sb[:], rhs=ht[:], start=True, stop=True
                    )
                    t0 = (w // 2) * 128
                    nc.vector.tensor_scalar_add(
                        out_sb[:, t0 : t0 + 128], z[:], b_sb[:]
                    )

            nc.sync.dma_start(out=t_out[:], in_=out_sb[:])

    nc.compile()
    return nc


def kernel(x, src, dst, W, b):
    x = np.asarray(x, dtype=np.float32)
    W = np.asarray(W, dtype=np.float32)
    b = np.asarray(b, dtype=np.float32)

    xlo, xhi, kA, kB, SA, SB, offA, offB, per_core = _prep(x, src, dst)
    nc = _build_program(kA, kB, SA, SB, offA, offB)

    wt = np.ascontiguousarray(W.T)
    bcol = np.ascontiguousarray(b.reshape(D, 1))
    iota_arr = np.tile(
        np.arange(WIN, dtype=np.float32)[None, :], (128, CHUNK)
    ).copy()
    ident_arr = np.eye(D, dtype=np.float32)

    in_maps = []
    for c in range(N_CORES):
        iA, dA, iB, dB = per_core[c]
        in_maps.append(
            {
                "xlo": xlo,
                "xhi": xhi,
                "wt": wt,
                "bias": bcol,
                "idxa": _wrap_idx(iA),
                "dsta": _wrap_dst(dA),
                "idxb": _wrap_idx(iB),
                "dstb": _wrap_dst(dB),
                "iota": iota_arr,
                "ident": ident_arr,
            }
        )

    res = run_bass_kernel_spmd(nc, in_maps, list(range(N_CORES)))
    LAST["results"] = res
    LAST["exec_time_ns"] = res.exec_time_ns

    out_t = np.concatenate([res.results[c]["out"] for c in range(N_CORES)], axis=1)
    return np.ascontiguousarray(out_t.T[:N_NODES])



# revision 3
# speedup vs baseline: 1.0394x; 1.0394x over previous
"""GCNConv mean-aggregation kernel for 8 Trainium2 NeuronCores.

Reference computation:
    msgs   = x[src]                       # [E, D] gather
    summed = segment_sum(msgs, dst, N)    # [N, D]
    deg    = segment_sum(ones, dst, N)    # [N]
    h      = summed / max(deg, 1)
    out    = h @ W.T + b                  # [N, D_OUT]

Strategy (no collectives needed):
  - Shard edges by contiguous dst ranges: core c owns nodes
    [c*6272, (c+1)*6272).  Each core fully reduces its own node range.
  - Gather tables hold x in bf16, 256B rows (64 feats + 64 zero pad) --
    256B is the dma_gather minimum element size, and bf16 operands run
    the PE at 4x the fp32 rate.
  - Per core the edge stream is grouped into 64-node dst windows.  For
    each 128-edge subtile we gather rows with big dma_gather calls
    rotated over the 4 SWDGE queues (single_packet=False), build a
    [128e, 64n] one-hot from dst via DVE is_equal (bf16 out), and
    accumulate  msgs.T @ onehot  into a [64, 64] PSUM tile: that is
    h.T directly -- no transpose pass needed.
  - Degree reciprocals are computed on host (they only depend on dst)
    and shipped replicated as [64, NPC]; normalization is one DVE
    multiply per window writing bf16 h.T slices.
  - Final dense layer: z = W @ h.T per 512-column tile (bf16 matmul),
    bias add on DVE, write out.T slices.  Host reassembles/transposes.
  - dma_gather indices are int16, so x is staged into two gather tables
    (src < 32767 and src >= 32767), each with a zero row at index 0
    used by padding edges (contributes 0 to sums).
"""

import sys

sys.path.insert(0, "/opt/trn_rl_repo")

import ml_dtypes
import numpy as np

import concourse.bacc as bacc
import concourse.mybir as mybir
import concourse.tile as tile
from concourse.bass_utils import run_bass_kernel_spmd

N_NODES = 50000
N_EDGES = 800000
D = 64
N_CORES = 8
NPC = 6272          # nodes per core (= 98 windows of 64)
WIN = 64            # dst-window width per PSUM accumulation group
N_WIN = NPC // WIN  # 98
SPLIT = 32767       # src < SPLIT -> lo table, else hi table
ROW = 128           # gather row: 64 bf16 feats + 64 bf16 zero pad (256 B)
CHUNK = 16          # subtiles (of 128 edges) per dma_gather call
NQ = 4              # SWDGE queues for parallel gather descriptor work
ZCOL = 512          # output columns per dense-layer matmul tile

F32 = mybir.dt.float32
BF16 = mybir.dt.bfloat16
I16 = mybir.dt.int16

BF = ml_dtypes.bfloat16

# Results of the most recent run (for test harness inspection).
LAST = {}


def _prep(x, src, dst):
    """Host-side sharding: build bf16 gather tables, per-core padded edge
    streams (int16 gather idx + f32 dst-rel), per-core replicated degree
    reciprocals, and per-window subtile budgets (shared across cores;
    SPMD program structure)."""
    x = np.asarray(x, dtype=np.float32)
    src = np.asarray(src, dtype=np.int64)
    dst = np.asarray(dst, dtype=np.int64)

    n_lo = SPLIT
    n_hi = N_NODES - SPLIT
    xlo = np.zeros((n_lo + 1, ROW), dtype=BF)
    xlo[1:, :D] = x[:SPLIT].astype(BF)
    xhi = np.zeros((n_hi + 1, ROW), dtype=BF)
    xhi[1:, :D] = x[SPLIT:].astype(BF)

    deg = np.bincount(dst, minlength=N_NODES).astype(np.float32)
    rec = np.ones(N_CORES * NPC, dtype=np.float32)
    rec[:N_NODES] = 1.0 / np.maximum(deg, 1.0)

    gw = (dst // WIN).astype(np.int64)
    cls = (src >= SPLIT).astype(np.int64)
    key = gw * 2 + cls
    order = np.argsort(key, kind="stable")
    src_s, dst_s = src[order], dst[order]

    n_groups = (N_CORES * N_WIN) * 2
    counts = np.bincount(key[order], minlength=n_groups)
    starts = np.zeros(n_groups + 1, dtype=np.int64)
    np.cumsum(counts, out=starts[1:])

    cnt = counts.reshape(N_CORES, N_WIN, 2)
    kA = np.maximum(1, -(-cnt[:, :, 0].max(axis=0) // 128))  # [N_WIN]
    kB = -(-cnt[:, :, 1].max(axis=0) // 128)                  # [N_WIN]
    SA = int(kA.sum())
    SB = int(kB.sum())

    idx_lo = (src_s + 1).astype(np.int16)
    idx_hi = (src_s - SPLIT + 1).astype(np.int16)

    offA = np.zeros(N_WIN + 1, dtype=np.int64)
    np.cumsum(kA, out=offA[1:])
    offB = np.zeros(N_WIN + 1, dtype=np.int64)
    np.cumsum(kB, out=offB[1:])

    per_core = []
    for c in range(N_CORES):
        iA = np.zeros(SA * 128, dtype=np.int16)
        dA = np.zeros(SA * 128, dtype=np.float32)
        iB = np.zeros(max(SB, 1) * 128, dtype=np.int16)
        dB = np.zeros(max(SB, 1) * 128, dtype=np.float32)
        for w in range(N_WIN):
            g = (c * N_WIN + w) * 2
            base = (c * N_WIN + w) * WIN
            s0, s1 = starts[g], starts[g + 1]
            p0 = int(offA[w]) * 128
            iA[p0 : p0 + (s1 - s0)] = idx_lo[s0:s1]
            dA[p0 : p0 + (s1 - s0)] = (dst_s[s0:s1] - base).astype(np.float32)
            s0, s1 = starts[g + 1], starts[g + 2]
            p0 = int(offB[w]) * 128
            iB[p0 : p0 + (s1 - s0)] = idx_hi[s0:s1]
            dB[p0 : p0 + (s1 - s0)] = (dst_s[s0:s1] - base).astype(np.float32)
        rec_c = np.tile(rec[c * NPC : (c + 1) * NPC][None, :], (D, 1))
        per_core.append((iA, dA, iB, dB, np.ascontiguousarray(rec_c)))

    return xlo, xhi, kA, kB, SA, SB, offA, offB, per_core


def _wrap_idx(idx_flat):
    """int16 stream -> dma_gather layout [128, n/16]: value i at
    [i % 16, i // 16], replicated across the 8 groups of 16 partitions."""
    a = idx_flat.reshape(-1, 16).T
    return np.tile(a, (8, 1)).copy()


def _wrap_dst(d_flat):
    """f32 stream -> [128, S]: subtile s lane e at [e, s]."""
    return np.ascontiguousarray(d_flat.reshape(-1, 128).T)


def _build_program(kA, kB, SA, SB, offA, offB):
    nc = bacc.Bacc(
        "TRN2", target_bir_lowering=False, debug=False, num_swdge_queues=NQ
    )

    t_xlo = nc.dram_tensor("xlo", [SPLIT + 1, ROW], BF16, kind="ExternalInput")
    t_xhi = nc.dram_tensor(
        "xhi", [N_NODES - SPLIT + 1, ROW], BF16, kind="ExternalInput"
    )
    t_wt = nc.dram_tensor("wt", [D, D], BF16, kind="ExternalInput")
    t_b = nc.dram_tensor("bias", [D, 1], F32, kind="ExternalInput")
    t_ia = nc.dram_tensor("idxa", [128, SA * 8], I16, kind="ExternalInput")
    t_da = nc.dram_tensor("dsta", [128, SA], F32, kind="ExternalInput")
    SBp = max(SB, 1)
    t_ib = nc.dram_tensor("idxb", [128, SBp * 8], I16, kind="ExternalInput")
    t_db = nc.dram_tensor("dstb", [128, SBp], F32, kind="ExternalInput")
    t_iota = nc.dram_tensor("iota", [128, CHUNK * WIN], F32, kind="ExternalInput")
    t_rec = nc.dram_tensor("rec", [D, NPC], F32, kind="ExternalInput")
    t_out = nc.dram_tensor("out", [D, NPC], F32, kind="ExternalOutput")

    callsA = [(p, min(CHUNK, SA - p)) for p in range(0, SA, CHUNK)]
    callsB = [(p, min(CHUNK, SB - p)) for p in range(0, SB, CHUNK)]

    with tile.TileContext(nc) as tc:
        with (
            tc.tile_pool(name="const", bufs=1) as cpool,
            tc.tile_pool(name="idx", bufs=1) as ipool,
            tc.tile_pool(name="msgsa", bufs=4) as mpa,
            tc.tile_pool(name="msgsb", bufs=3) as mpb,
            tc.tile_pool(name="oha", bufs=4) as opa,
            tc.tile_pool(name="ohb", bufs=3) as opb,
            tc.tile_pool(name="psacc", bufs=4, space="PSUM") as ps_acc,
            tc.tile_pool(name="psz", bufs=2, space="PSUM") as ps_z,
        ):
            wt_sb = cpool.tile([D, D], BF16)
            nc.sync.dma_start(out=wt_sb[:], in_=t_wt[:])
            b_sb = cpool.tile([D, 1], F32)
            nc.sync.dma_start(out=b_sb[:], in_=t_b[:])
            iota_f = cpool.tile([128, CHUNK * WIN], F32)
            nc.sync.dma_start(out=iota_f[:], in_=t_iota[:])
            rec_sb = cpool.tile([D, NPC], F32)
            nc.sync.dma_start(out=rec_sb[:], in_=t_rec[:])

            ia_sb = ipool.tile([128, SA * 8], I16)
            nc.sync.dma_start(out=ia_sb[:], in_=t_ia[:])
            da_sb = ipool.tile([128, SA], F32)
            nc.sync.dma_start(out=da_sb[:], in_=t_da[:])
            ib_sb = ipool.tile([128, SBp * 8], I16)
            nc.sync.dma_start(out=ib_sb[:], in_=t_ib[:])
            db_sb = ipool.tile([128, SBp], F32)
            nc.sync.dma_start(out=db_sb[:], in_=t_db[:])

            ht_sb = cpool.tile([D, NPC], BF16)
            out_sb = cpool.tile([D, NPC], F32)

            chunk_tiles = {0: [], 1: []}
            call_no = [0]

            def emit_chunk(st, k):
                if st == 0:
                    pos, nsub = callsA[k]
                    mp, op, tsrc, isb, dsb = mpa, opa, t_xlo, ia_sb, da_sb
                else:
                    pos, nsub = callsB[k]
                    mp, op, tsrc, isb, dsb = mpb, opb, t_xhi, ib_sb, db_sb
                msgs = mp.tile([128, CHUNK, ROW], BF16)
                nidx = nsub * 128
                # single_packet=False: one packet per descriptor. The default
                # coalesces the whole call into one SDMA packet, which wedges
                # the engine beyond ~64 descriptors/lane (num_idxs >~ 1000).
                # Rotating queue_num spreads descriptor generation + ring
                # drain over the 4 SWDGE queues (~2x measured).
                nc.gpsimd.dma_gather(
                    msgs[:, :nsub, :],
                    tsrc[:],
                    isb[:, pos * 8 : pos * 8 + nsub * 8],
                    nidx,
                    nidx,
                    ROW,
                    single_packet=False,
                    queue_num=call_no[0] % NQ,
                )
                call_no[0] += 1
                oh = op.tile([128, CHUNK * WIN], BF16)
                dst_b = (
                    dsb[:, pos : pos + nsub]
                    .unsqueeze(2)
                    .to_broadcast([128, nsub, WIN])
                )
                nc.vector.tensor_tensor(
                    out=oh[:, : nsub * WIN].rearrange("p (s w) -> p s w", w=WIN),
                    in0=iota_f[:, : nsub * WIN].rearrange(
                        "p (s w) -> p s w", w=WIN
                    ),
                    in1=dst_b,
                    op=mybir.AluOpType.is_equal,
                )
                chunk_tiles[st].append((msgs, oh))

            cursor = [0, 0]

            def tiles_for(st, s):
                k = s // CHUNK
                while cursor[st] <= k:
                    emit_chunk(st, cursor[st])
                    cursor[st] += 1
                msgs, oh = chunk_tiles[st][k]
                return msgs, oh, s % CHUNK

            for w in range(N_WIN):
                subs = [(0, int(offA[w]) + j) for j in range(int(kA[w]))]
                subs += [(1, int(offB[w]) + j) for j in range(int(kB[w]))]
                ps = ps_acc.tile([D, WIN], F32)
                for j, (st, s) in enumerate(subs):
                    msgs, oh, col = tiles_for(st, s)
                    nc.tensor.matmul(
                        out=ps[:],
                        lhsT=msgs[:, col, :D],
                        rhs=oh[:, col * WIN : (col + 1) * WIN],
                        start=(j == 0),
                        stop=(j == len(subs) - 1),
                    )
                # normalize: h.T slice = ps * (1/deg), written as bf16
                nc.vector.tensor_tensor(
                    out=ht_sb[:, w * WIN : (w + 1) * WIN],
                    in0=ps[:],
                    in1=rec_sb[:, w * WIN : (w + 1) * WIN],
                    op=mybir.AluOpType.mult,
                )
                # dense layer every ZCOL finished columns
                t0 = (w // (ZCOL // WIN)) * ZCOL
                done = (w + 1) * WIN
                if done - t0 == ZCOL or w == N_WIN - 1:
                    zc = done - t0
                    z = ps_z.tile([D, ZCOL], F32)
                    nc.tensor.matmul(
                        out=z[:, :zc],
                        lhsT=wt_sb[:],
                        rhs=ht_sb[:, t0 : t0 + zc],
                        start=True,
                        stop=True,
                    )
                    nc.vector.tensor_scalar_add(
                        out_sb[:, t0 : t0 + zc], z[:, :zc], b_sb[:]
                    )

            nc.sync.dma_start(out=t_out[:], in_=out_sb[:])

    nc.compile()
    return nc


def kernel(x, src, dst, W, b):
    x = np.asarray(x, dtype=np.float32)
    W = np.asarray(W, dtype=np.float32)
    b = np.asarray(b, dtype=np.float32)

    xlo, xhi, kA, kB, SA, SB, offA, offB, per_core = _prep(x, src, dst)
    nc = _build_program(kA, kB, SA, SB, offA, offB)

    wt = np.ascontiguousarray(W.T).astype(BF)
    bcol = np.ascontiguousarray(b.reshape(D, 1))
    iota_arr = np.tile(
        np.arange(WIN, dtype=np.float32)[None, :], (128, CHUNK)
    ).copy()

    in_maps = []
    for c in range(N_CORES):
        iA, dA, iB, dB, rec_c = per_core[c]
        in_maps.append(
            {
                "xlo": xlo,
                "xhi": xhi,
                "wt": wt,
                "bias": bcol,
                "idxa": _wrap_idx(iA),
                "dsta": _wrap_dst(dA),
                "idxb": _wrap_idx(iB),
                "dstb": _wrap_dst(dB),
                "iota": iota_arr,
                "rec": rec_c,
            }
        )

    res = run_bass_kernel_spmd(nc, in_maps, list(range(N_CORES)))
    LAST["results"] = res
    LAST["exec_time_ns"] = res.exec_time_ns

    out_t = np.concatenate([res.results[c]["out"] for c in range(N_CORES)], axis=1)
    return np.ascontiguousarray(out_t.T[:N_NODES])


# revision 5
# speedup vs baseline: 1.1720x; 1.1276x over previous
"""GCNConv mean-aggregation kernel for 8 Trainium2 NeuronCores.

Reference computation:
    msgs   = x[src]                       # [E, D] gather
    summed = segment_sum(msgs, dst, N)    # [N, D]
    deg    = segment_sum(ones, dst, N)    # [N]
    h      = summed / max(deg, 1)
    out    = h @ W.T + b                  # [N, D_OUT]

Strategy (no collectives needed):
  - Shard edges by contiguous dst ranges: core c owns nodes
    [c*6272, (c+1)*6272).  Each core fully reduces its own node range.
  - Gather tables hold x in bf16, 256B rows (64 feats + 64 zero pad) --
    256B is the dma_gather minimum element size, and bf16 operands run
    the PE at 4x the fp32 rate.
  - Per core the edge stream is grouped into 64-node dst windows.  For
    each 128-edge subtile we gather rows with big dma_gather calls
    rotated over the 4 SWDGE queues (single_packet=False), build a
    [128e, 64n] one-hot from dst via DVE is_equal (bf16 out), and
    accumulate  msgs.T @ onehot  into a [64, 64] PSUM tile: that is
    h.T directly -- no transpose pass needed.
  - Degree reciprocals are computed on host (they only depend on dst)
    and shipped replicated as [64, NPC]; normalization is one DVE
    multiply per window writing bf16 h.T slices.
  - Final dense layer: z = W @ h.T per 512-column tile (bf16 matmul),
    bias add on DVE, write out.T slices.  Host reassembles/transposes.
  - dma_gather indices are int16, so x is staged into two gather tables
    (src < 32767 and src >= 32767), each with a zero row at index 0
    used by padding edges (contributes 0 to sums).
"""

import sys

sys.path.insert(0, "/opt/trn_rl_repo")

import ml_dtypes
import numpy as np

import concourse.bacc as bacc
import concourse.mybir as mybir
import concourse.tile as tile
from concourse.bass_utils import run_bass_kernel_spmd

N_NODES = 50000
N_EDGES = 800000
D = 64
N_CORES = 8
NPC = 6272          # nodes per core (= 98 windows of 64)
WIN = 64            # dst-window width per PSUM accumulation group
N_WIN = NPC // WIN  # 98
SPLIT = 32767       # src < SPLIT -> lo table, else hi table
ROW = 128           # gather row: 64 bf16 feats + 64 bf16 zero pad (256 B)
CHUNK = 7           # subtiles (of 128 edges) per dma_gather call
NQ = 4              # SWDGE queues for parallel gather descriptor work
ZCOL = 512          # output columns per dense-layer matmul tile

F32 = mybir.dt.float32
BF16 = mybir.dt.bfloat16
I16 = mybir.dt.int16

BF = ml_dtypes.bfloat16

# Results of the most recent run (for test harness inspection).
LAST = {}


def _prep(x, src, dst):
    """Host-side sharding: build bf16 gather tables, per-core padded edge
    streams (int16 gather idx + f32 dst-rel), per-core replicated degree
    reciprocals, and per-window subtile budgets (shared across cores;
    SPMD program structure)."""
    x = np.asarray(x, dtype=np.float32)
    src = np.asarray(src, dtype=np.int64)
    dst = np.asarray(dst, dtype=np.int64)

    n_lo = SPLIT
    n_hi = N_NODES - SPLIT
    xlo = np.zeros((n_lo + 1, ROW), dtype=BF)
    xlo[1:, :D] = x[:SPLIT].astype(BF)
    xhi = np.zeros((n_hi + 1, ROW), dtype=BF)
    xhi[1:, :D] = x[SPLIT:].astype(BF)

    deg = np.bincount(dst, minlength=N_NODES).astype(np.float32)
    rec = np.ones(N_CORES * NPC, dtype=np.float32)
    rec[:N_NODES] = 1.0 / np.maximum(deg, 1.0)

    gw = (dst // WIN).astype(np.int64)
    cls = (src >= SPLIT).astype(np.int64)
    key = gw * 2 + cls
    order = np.argsort(key, kind="stable")
    src_s, dst_s = src[order], dst[order]

    n_groups = (N_CORES * N_WIN) * 2
    counts = np.bincount(key[order], minlength=n_groups)
    starts = np.zeros(n_groups + 1, dtype=np.int64)
    np.cumsum(counts, out=starts[1:])

    cnt = counts.reshape(N_CORES, N_WIN, 2)
    kA = np.maximum(1, -(-cnt[:, :, 0].max(axis=0) // 128))  # [N_WIN]
    kB = -(-cnt[:, :, 1].max(axis=0) // 128)                  # [N_WIN]
    SA = int(kA.sum())
    SB = int(kB.sum())

    idx_lo = (src_s + 1).astype(np.int16)
    idx_hi = (src_s - SPLIT + 1).astype(np.int16)

    offA = np.zeros(N_WIN + 1, dtype=np.int64)
    np.cumsum(kA, out=offA[1:])
    offB = np.zeros(N_WIN + 1, dtype=np.int64)
    np.cumsum(kB, out=offB[1:])

    per_core = []
    for c in range(N_CORES):
        iA = np.zeros(SA * 128, dtype=np.int16)
        dA = np.zeros(SA * 128, dtype=np.float32)
        iB = np.zeros(max(SB, 1) * 128, dtype=np.int16)
        dB = np.zeros(max(SB, 1) * 128, dtype=np.float32)
        for w in range(N_WIN):
            g = (c * N_WIN + w) * 2
            base = (c * N_WIN + w) * WIN
            s0, s1 = starts[g], starts[g + 1]
            p0 = int(offA[w]) * 128
            iA[p0 : p0 + (s1 - s0)] = idx_lo[s0:s1]
            dA[p0 : p0 + (s1 - s0)] = (dst_s[s0:s1] - base).astype(np.float32)
            s0, s1 = starts[g + 1], starts[g + 2]
            p0 = int(offB[w]) * 128
            iB[p0 : p0 + (s1 - s0)] = idx_hi[s0:s1]
            dB[p0 : p0 + (s1 - s0)] = (dst_s[s0:s1] - base).astype(np.float32)
        rec_c = np.tile(rec[c * NPC : (c + 1) * NPC][None, :], (D, 1))
        per_core.append((iA, dA, iB, dB, np.ascontiguousarray(rec_c)))

    return xlo, xhi, kA, kB, SA, SB, offA, offB, per_core


def _wrap_idx(idx_flat):
    """int16 stream -> dma_gather layout [128, n/16]: value i at
    [i % 16, i // 16], replicated across the 8 groups of 16 partitions."""
    a = idx_flat.reshape(-1, 16).T
    return np.tile(a, (8, 1)).copy()


def _wrap_dst(d_flat):
    """f32 stream -> [128, S]: subtile s lane e at [e, s]."""
    return np.ascontiguousarray(d_flat.reshape(-1, 128).T)


def _build_program(kA, kB, SA, SB, offA, offB):
    nc = bacc.Bacc(
        "TRN2", target_bir_lowering=False, debug=False, num_swdge_queues=NQ
    )

    t_xlo = nc.dram_tensor("xlo", [SPLIT + 1, ROW], BF16, kind="ExternalInput")
    t_xhi = nc.dram_tensor(
        "xhi", [N_NODES - SPLIT + 1, ROW], BF16, kind="ExternalInput"
    )
    t_wt = nc.dram_tensor("wt", [D, D], BF16, kind="ExternalInput")
    t_b = nc.dram_tensor("bias", [D, 1], F32, kind="ExternalInput")
    t_ia = nc.dram_tensor("idxa", [128, SA * 8], I16, kind="ExternalInput")
    t_da = nc.dram_tensor("dsta", [128, SA], F32, kind="ExternalInput")
    SBp = max(SB, 1)
    t_ib = nc.dram_tensor("idxb", [128, SBp * 8], I16, kind="ExternalInput")
    t_db = nc.dram_tensor("dstb", [128, SBp], F32, kind="ExternalInput")
    t_iota = nc.dram_tensor("iota", [128, CHUNK * WIN], F32, kind="ExternalInput")
    t_rec = nc.dram_tensor("rec", [D, NPC], F32, kind="ExternalInput")
    t_out = nc.dram_tensor("out", [D, NPC], F32, kind="ExternalOutput")

    callsA = [(p, min(CHUNK, SA - p)) for p in range(0, SA, CHUNK)]
    callsB = [(p, min(CHUNK, SB - p)) for p in range(0, SB, CHUNK)]

    with tile.TileContext(nc) as tc:
        with (
            tc.tile_pool(name="const", bufs=1) as cpool,
            tc.tile_pool(name="idx", bufs=1) as ipool,
            tc.tile_pool(name="msgsa", bufs=4) as mpa,
            tc.tile_pool(name="msgsb", bufs=3) as mpb,
            tc.tile_pool(name="oha", bufs=4) as opa,
            tc.tile_pool(name="ohb", bufs=3) as opb,
            tc.tile_pool(name="psacc", bufs=4, space="PSUM") as ps_acc,
            tc.tile_pool(name="psz", bufs=2, space="PSUM") as ps_z,
        ):
            wt_sb = cpool.tile([D, D], BF16)
            nc.sync.dma_start(out=wt_sb[:], in_=t_wt[:])
            b_sb = cpool.tile([D, 1], F32)
            nc.sync.dma_start(out=b_sb[:], in_=t_b[:])
            iota_f = cpool.tile([128, CHUNK * WIN], F32)
            nc.sync.dma_start(out=iota_f[:], in_=t_iota[:])
            rec_sb = cpool.tile([D, NPC], F32)
            nc.sync.dma_start(out=rec_sb[:], in_=t_rec[:])

            ia_sb = ipool.tile([128, SA * 8], I16)
            nc.sync.dma_start(out=ia_sb[:], in_=t_ia[:])
            da_sb = ipool.tile([128, SA], F32)
            nc.sync.dma_start(out=da_sb[:], in_=t_da[:])
            ib_sb = ipool.tile([128, SBp * 8], I16)
            nc.sync.dma_start(out=ib_sb[:], in_=t_ib[:])
            db_sb = ipool.tile([128, SBp], F32)
            nc.sync.dma_start(out=db_sb[:], in_=t_db[:])

            ht_sb = cpool.tile([D, NPC], BF16)
            out_sb = cpool.tile([D, NPC], F32)

            chunk_tiles = {0: [], 1: []}
            call_no = [0]

            def emit_chunk(st, k):
                if st == 0:
                    pos, nsub = callsA[k]
                    mp, op, tsrc, isb, dsb = mpa, opa, t_xlo, ia_sb, da_sb
                else:
                    pos, nsub = callsB[k]
                    mp, op, tsrc, isb, dsb = mpb, opb, t_xhi, ib_sb, db_sb
                msgs = mp.tile([128, CHUNK, ROW], BF16)
                nidx = nsub * 128
                # single_packet=True coalesces the call into one SDMA packet,
                # dodging the ~2.4ns/desc GPSIMD packet-write cost of the
                # per-descriptor mode.  The engine wedges when a coalesced
                # call exceeds ~1000 descriptors, so calls stay under that.
                # Rotating queue_num spreads descriptor generation + ring
                # drain over the 4 SWDGE queues.
                nc.gpsimd.dma_gather(
                    msgs[:, :nsub, :],
                    tsrc[:],
                    isb[:, pos * 8 : pos * 8 + nsub * 8],
                    nidx,
                    nidx,
                    ROW,
                    single_packet=True,
                    queue_num=call_no[0] % NQ,
                )
                call_no[0] += 1
                oh = op.tile([128, CHUNK * WIN], BF16)
                dst_b = (
                    dsb[:, pos : pos + nsub]
                    .unsqueeze(2)
                    .to_broadcast([128, nsub, WIN])
                )
                nc.vector.tensor_tensor(
                    out=oh[:, : nsub * WIN].rearrange("p (s w) -> p s w", w=WIN),
                    in0=iota_f[:, : nsub * WIN].rearrange(
                        "p (s w) -> p s w", w=WIN
                    ),
                    in1=dst_b,
                    op=mybir.AluOpType.is_equal,
                )
                chunk_tiles[st].append((msgs, oh))

            cursor = [0, 0]

            def tiles_for(st, s):
                k = s // CHUNK
                while cursor[st] <= k:
                    emit_chunk(st, cursor[st])
                    cursor[st] += 1
                msgs, oh = chunk_tiles[st][k]
                return msgs, oh, s % CHUNK

            for w in range(N_WIN):
                subs = [(0, int(offA[w]) + j) for j in range(int(kA[w]))]
                subs += [(1, int(offB[w]) + j) for j in range(int(kB[w]))]
                ps = ps_acc.tile([D, WIN], F32)
                for j, (st, s) in enumerate(subs):
                    msgs, oh, col = tiles_for(st, s)
                    nc.tensor.matmul(
                        out=ps[:],
                        lhsT=msgs[:, col, :D],
                        rhs=oh[:, col * WIN : (col + 1) * WIN],
                        start=(j == 0),
                        stop=(j == len(subs) - 1),
                    )
                # normalize: h.T slice = ps * (1/deg), written as bf16
                nc.vector.tensor_tensor(
                    out=ht_sb[:, w * WIN : (w + 1) * WIN],
                    in0=ps[:],
                    in1=rec_sb[:, w * WIN : (w + 1) * WIN],
                    op=mybir.AluOpType.mult,
                )
                # dense layer every ZCOL finished columns
                t0 = (w // (ZCOL // WIN)) * ZCOL
                done = (w + 1) * WIN
                if done - t0 == ZCOL or w == N_WIN - 1:
                    zc = done - t0
                    z = ps_z.tile([D, ZCOL], F32)
                    nc.tensor.matmul(
                        out=z[:, :zc],
                        lhsT=wt_sb[:],
                        rhs=ht_sb[:, t0 : t0 + zc],
                        start=True,
                        stop=True,
                    )
                    nc.vector.tensor_scalar_add(
                        out_sb[:, t0 : t0 + zc], z[:, :zc], b_sb[:]
                    )

            nc.sync.dma_start(out=t_out[:], in_=out_sb[:])

    nc.compile()
    return nc


def kernel(x, src, dst, W, b):
    x = np.asarray(x, dtype=np.float32)
    W = np.asarray(W, dtype=np.float32)
    b = np.asarray(b, dtype=np.float32)

    xlo, xhi, kA, kB, SA, SB, offA, offB, per_core = _prep(x, src, dst)
    nc = _build_program(kA, kB, SA, SB, offA, offB)

    wt = np.ascontiguousarray(W.T).astype(BF)
    bcol = np.ascontiguousarray(b.reshape(D, 1))
    iota_arr = np.tile(
        np.arange(WIN, dtype=np.float32)[None, :], (128, CHUNK)
    ).copy()

    in_maps = []
    for c in range(N_CORES):
        iA, dA, iB, dB, rec_c = per_core[c]
        in_maps.append(
            {
                "xlo": xlo,
                "xhi": xhi,
                "wt": wt,
                "bias": bcol,
                "idxa": _wrap_idx(iA),
                "dsta": _wrap_dst(dA),
                "idxb": _wrap_idx(iB),
                "dstb": _wrap_dst(dB),
                "iota": iota_arr,
                "rec": rec_c,
            }
        )

    res = run_bass_kernel_spmd(nc, in_maps, list(range(N_CORES)))
    LAST["results"] = res
    LAST["exec_time_ns"] = res.exec_time_ns

    out_t = np.concatenate([res.results[c]["out"] for c in range(N_CORES)], axis=1)
    return np.ascontiguousarray(out_t.T[:N_NODES])


# revision 7
# speedup vs baseline: 1.2267x; 1.0467x over previous
"""GCNConv mean-aggregation kernel for 8 Trainium2 NeuronCores.

Reference computation:
    msgs   = x[src]                       # [E, D] gather
    summed = segment_sum(msgs, dst, N)    # [N, D]
    deg    = segment_sum(ones, dst, N)    # [N]
    h      = summed / max(deg, 1)
    out    = h @ W.T + b                  # [N, D_OUT]

Strategy (no collectives needed):
  - Shard edges by contiguous dst ranges: core c owns nodes
    [c*6272, (c+1)*6272).  Each core fully reduces its own node range.
  - Gather tables hold x in bf16, 256B rows (64 feats + 64 zero pad) --
    256B is the dma_gather minimum element size, and bf16 operands run
    the PE at 4x the fp32 rate.
  - Per core the edge stream is grouped into 64-node dst windows.  For
    each 128-edge subtile we gather rows with big dma_gather calls
    rotated over the 4 SWDGE queues (single_packet=False), build a
    [128e, 64n] one-hot from dst via DVE is_equal (bf16 out), and
    accumulate  msgs.T @ onehot  into a [64, 64] PSUM tile: that is
    h.T directly -- no transpose pass needed.
  - Degree reciprocals are computed on host (they only depend on dst)
    and shipped replicated as [64, NPC]; normalization is one DVE
    multiply per window writing bf16 h.T slices.
  - Final dense layer: z = W @ h.T per 512-column tile (bf16 matmul),
    bias add on DVE, write out.T slices.  Host reassembles/transposes.
  - dma_gather indices are int16, so x is staged into two gather tables
    (src < 32767 and src >= 32767), each with a zero row at index 0
    used by padding edges (contributes 0 to sums).
"""

import sys

sys.path.insert(0, "/opt/trn_rl_repo")

import ml_dtypes
import numpy as np

import concourse.bacc as bacc
import concourse.mybir as mybir
import concourse.tile as tile
from concourse.bass_utils import run_bass_kernel_spmd

N_NODES = 50000
N_EDGES = 800000
D = 64
N_CORES = 8
NPC = 6272          # nodes per core (= 98 windows of 64)
WIN = 64            # dst-window width per PSUM accumulation group
N_WIN = NPC // WIN  # 98
SPLIT = 32767       # src < SPLIT -> lo table, else hi table
ROW = 128           # gather row: 64 bf16 feats + 64 bf16 zero pad (256 B)
CHUNK = 8           # subtiles (of 128 edges) per dma_gather call
NQ = 4              # SWDGE queues for parallel gather descriptor work
ZCOL = 512          # output columns per dense-layer matmul tile

F32 = mybir.dt.float32
BF16 = mybir.dt.bfloat16
I16 = mybir.dt.int16

BF = ml_dtypes.bfloat16

# Results of the most recent run (for test harness inspection).
LAST = {}


def _prep(x, src, dst):
    """Host-side sharding: build bf16 gather tables, per-core padded edge
    streams (int16 gather idx + f32 dst-rel), per-core replicated degree
    reciprocals, and per-window subtile budgets (shared across cores;
    SPMD program structure)."""
    x = np.asarray(x, dtype=np.float32)
    src = np.asarray(src, dtype=np.int64)
    dst = np.asarray(dst, dtype=np.int64)

    n_lo = SPLIT
    n_hi = N_NODES - SPLIT
    xlo = np.zeros((n_lo + 1, ROW), dtype=BF)
    xlo[1:, :D] = x[:SPLIT].astype(BF)
    xhi = np.zeros((n_hi + 1, ROW), dtype=BF)
    xhi[1:, :D] = x[SPLIT:].astype(BF)

    deg = np.bincount(dst, minlength=N_NODES).astype(np.float32)
    rec = np.ones(N_CORES * NPC, dtype=np.float32)
    rec[:N_NODES] = 1.0 / np.maximum(deg, 1.0)

    gw = (dst // WIN).astype(np.int64)
    cls = (src >= SPLIT).astype(np.int64)
    key = gw * 2 + cls
    order = np.argsort(key, kind="stable")
    src_s, dst_s = src[order], dst[order]

    n_groups = (N_CORES * N_WIN) * 2
    counts = np.bincount(key[order], minlength=n_groups)
    starts = np.zeros(n_groups + 1, dtype=np.int64)
    np.cumsum(counts, out=starts[1:])

    cnt = counts.reshape(N_CORES, N_WIN, 2)
    kA = np.maximum(1, -(-cnt[:, :, 0].max(axis=0) // 128))  # [N_WIN]
    kB = -(-cnt[:, :, 1].max(axis=0) // 128)                  # [N_WIN]
    SA = int(kA.sum())
    SB = int(kB.sum())

    idx_lo = (src_s + 1).astype(np.int16)
    idx_hi = (src_s - SPLIT + 1).astype(np.int16)

    offA = np.zeros(N_WIN + 1, dtype=np.int64)
    np.cumsum(kA, out=offA[1:])
    offB = np.zeros(N_WIN + 1, dtype=np.int64)
    np.cumsum(kB, out=offB[1:])

    per_core = []
    for c in range(N_CORES):
        iA = np.zeros(SA * 128, dtype=np.int16)
        dA = np.zeros(SA * 128, dtype=np.float32)
        iB = np.zeros(max(SB, 1) * 128, dtype=np.int16)
        dB = np.zeros(max(SB, 1) * 128, dtype=np.float32)
        for w in range(N_WIN):
            g = (c * N_WIN + w) * 2
            base = (c * N_WIN + w) * WIN
            s0, s1 = starts[g], starts[g + 1]
            p0 = int(offA[w]) * 128
            iA[p0 : p0 + (s1 - s0)] = idx_lo[s0:s1]
            dA[p0 : p0 + (s1 - s0)] = (dst_s[s0:s1] - base).astype(np.float32)
            s0, s1 = starts[g + 1], starts[g + 2]
            p0 = int(offB[w]) * 128
            iB[p0 : p0 + (s1 - s0)] = idx_hi[s0:s1]
            dB[p0 : p0 + (s1 - s0)] = (dst_s[s0:s1] - base).astype(np.float32)
        rec_c = np.tile(rec[c * NPC : (c + 1) * NPC][None, :], (D, 1))
        per_core.append((iA, dA, iB, dB, np.ascontiguousarray(rec_c)))

    return xlo, xhi, kA, kB, SA, SB, offA, offB, per_core


def _wrap_idx(idx_flat):
    """int16 stream -> dma_gather layout [128, n/16]: value i at
    [i % 16, i // 16], replicated across the 8 groups of 16 partitions."""
    a = idx_flat.reshape(-1, 16).T
    return np.tile(a, (8, 1)).copy()


def _wrap_dst(d_flat):
    """f32 stream -> [128, S]: subtile s lane e at [e, s]."""
    return np.ascontiguousarray(d_flat.reshape(-1, 128).T)


def _build_program(kA, kB, SA, SB, offA, offB):
    nc = bacc.Bacc(
        "TRN2", target_bir_lowering=False, debug=False, num_swdge_queues=NQ
    )

    t_xlo = nc.dram_tensor("xlo", [SPLIT + 1, ROW], BF16, kind="ExternalInput")
    t_xhi = nc.dram_tensor(
        "xhi", [N_NODES - SPLIT + 1, ROW], BF16, kind="ExternalInput"
    )
    t_wt = nc.dram_tensor("wt", [D, D], BF16, kind="ExternalInput")
    t_b = nc.dram_tensor("bias", [D, 1], F32, kind="ExternalInput")
    t_ia = nc.dram_tensor("idxa", [128, SA * 8], I16, kind="ExternalInput")
    t_da = nc.dram_tensor("dsta", [128, SA], F32, kind="ExternalInput")
    SBp = max(SB, 1)
    t_ib = nc.dram_tensor("idxb", [128, SBp * 8], I16, kind="ExternalInput")
    t_db = nc.dram_tensor("dstb", [128, SBp], F32, kind="ExternalInput")
    t_iota = nc.dram_tensor("iota", [128, CHUNK * WIN], F32, kind="ExternalInput")
    t_rec = nc.dram_tensor("rec", [D, NPC], F32, kind="ExternalInput")
    t_out = nc.dram_tensor("out", [D, NPC], F32, kind="ExternalOutput")

    callsA = [(p, min(CHUNK, SA - p)) for p in range(0, SA, CHUNK)]
    callsB = [(p, min(CHUNK, SB - p)) for p in range(0, SB, CHUNK)]

    with tile.TileContext(nc) as tc:
        with (
            tc.tile_pool(name="const", bufs=1) as cpool,
            tc.tile_pool(name="idx", bufs=1) as ipool,
            tc.tile_pool(name="msgsa", bufs=4) as mpa,
            tc.tile_pool(name="msgsb", bufs=3) as mpb,
            tc.tile_pool(name="oha", bufs=4) as opa,
            tc.tile_pool(name="ohb", bufs=3) as opb,
            tc.tile_pool(name="psacc", bufs=4, space="PSUM") as ps_acc,
            tc.tile_pool(name="psz", bufs=2, space="PSUM") as ps_z,
        ):
            wt_sb = cpool.tile([D, D], BF16)
            nc.sync.dma_start(out=wt_sb[:], in_=t_wt[:])
            b_sb = cpool.tile([D, 1], F32)
            nc.sync.dma_start(out=b_sb[:], in_=t_b[:])
            iota_f = cpool.tile([128, CHUNK * WIN], F32)
            nc.sync.dma_start(out=iota_f[:], in_=t_iota[:])
            rec_sb = cpool.tile([D, NPC], F32)
            nc.sync.dma_start(out=rec_sb[:], in_=t_rec[:])

            ia_sb = ipool.tile([128, SA * 8], I16)
            nc.sync.dma_start(out=ia_sb[:], in_=t_ia[:])
            da_sb = ipool.tile([128, SA], F32)
            nc.sync.dma_start(out=da_sb[:], in_=t_da[:])
            ib_sb = ipool.tile([128, SBp * 8], I16)
            nc.sync.dma_start(out=ib_sb[:], in_=t_ib[:])
            db_sb = ipool.tile([128, SBp], F32)
            nc.sync.dma_start(out=db_sb[:], in_=t_db[:])

            ht_sb = cpool.tile([D, NPC], BF16)
            out_sb = cpool.tile([D, NPC], F32)

            chunk_tiles = {0: [], 1: []}
            call_no = [0]

            def emit_chunk(st, k):
                if st == 0:
                    pos, nsub = callsA[k]
                    mp, op, tsrc, isb, dsb = mpa, opa, t_xlo, ia_sb, da_sb
                else:
                    pos, nsub = callsB[k]
                    mp, op, tsrc, isb, dsb = mpb, opb, t_xhi, ib_sb, db_sb
                msgs = mp.tile([128, CHUNK, ROW], BF16)
                nidx = nsub * 128
                # single_packet=True coalesces the call into one SDMA packet,
                # dodging the ~2.4ns/desc GPSIMD packet-write cost of the
                # per-descriptor mode.  The engine wedges when a coalesced
                # call exceeds ~1000 descriptors, so calls stay under that.
                # Rotating queue_num spreads descriptor generation + ring
                # drain over the 4 SWDGE queues.
                nc.gpsimd.dma_gather(
                    msgs[:, :nsub, :],
                    tsrc[:],
                    isb[:, pos * 8 : pos * 8 + nsub * 8],
                    nidx,
                    nidx,
                    ROW,
                    single_packet=True,
                    queue_num=call_no[0] % NQ,
                )
                call_no[0] += 1
                oh = op.tile([128, CHUNK * WIN], BF16)
                dst_b = (
                    dsb[:, pos : pos + nsub]
                    .unsqueeze(2)
                    .to_broadcast([128, nsub, WIN])
                )
                nc.vector.tensor_tensor(
                    out=oh[:, : nsub * WIN].rearrange("p (s w) -> p s w", w=WIN),
                    in0=iota_f[:, : nsub * WIN].rearrange(
                        "p (s w) -> p s w", w=WIN
                    ),
                    in1=dst_b,
                    op=mybir.AluOpType.is_equal,
                )
                chunk_tiles[st].append((msgs, oh))

            cursor = [0, 0]

            def tiles_for(st, s):
                k = s // CHUNK
                while cursor[st] <= k:
                    emit_chunk(st, cursor[st])
                    cursor[st] += 1
                msgs, oh = chunk_tiles[st][k]
                return msgs, oh, s % CHUNK

            for w in range(N_WIN):
                subs = [(0, int(offA[w]) + j) for j in range(int(kA[w]))]
                subs += [(1, int(offB[w]) + j) for j in range(int(kB[w]))]
                ps = ps_acc.tile([D, WIN], F32)
                for j, (st, s) in enumerate(subs):
                    msgs, oh, col = tiles_for(st, s)
                    nc.tensor.matmul(
                        out=ps[:],
                        lhsT=msgs[:, col, :D],
                        rhs=oh[:, col * WIN : (col + 1) * WIN],
                        start=(j == 0),
                        stop=(j == len(subs) - 1),
                    )
                # normalize: h.T slice = ps * (1/deg), written as bf16
                nc.vector.tensor_tensor(
                    out=ht_sb[:, w * WIN : (w + 1) * WIN],
                    in0=ps[:],
                    in1=rec_sb[:, w * WIN : (w + 1) * WIN],
                    op=mybir.AluOpType.mult,
                )
                # dense layer every ZCOL finished columns
                t0 = (w // (ZCOL // WIN)) * ZCOL
                done = (w + 1) * WIN
                if done - t0 == ZCOL or w == N_WIN - 1:
                    zc = done - t0
                    z = ps_z.tile([D, ZCOL], F32)
                    nc.tensor.matmul(
                        out=z[:, :zc],
                        lhsT=wt_sb[:],
                        rhs=ht_sb[:, t0 : t0 + zc],
                        start=True,
                        stop=True,
                    )
                    nc.vector.tensor_scalar_add(
                        out_sb[:, t0 : t0 + zc], z[:, :zc], b_sb[:]
                    )

            nc.sync.dma_start(out=t_out[:], in_=out_sb[:])

    nc.compile()
    return nc


def kernel(x, src, dst, W, b):
    x = np.asarray(x, dtype=np.float32)
    W = np.asarray(W, dtype=np.float32)
    b = np.asarray(b, dtype=np.float32)

    xlo, xhi, kA, kB, SA, SB, offA, offB, per_core = _prep(x, src, dst)
    nc = _build_program(kA, kB, SA, SB, offA, offB)

    wt = np.ascontiguousarray(W.T).astype(BF)
    bcol = np.ascontiguousarray(b.reshape(D, 1))
    iota_arr = np.tile(
        np.arange(WIN, dtype=np.float32)[None, :], (128, CHUNK)
    ).copy()

    in_maps = []
    for c in range(N_CORES):
        iA, dA, iB, dB, rec_c = per_core[c]
        in_maps.append(
            {
                "xlo": xlo,
                "xhi": xhi,
                "wt": wt,
                "bias": bcol,
                "idxa": _wrap_idx(iA),
                "dsta": _wrap_dst(dA),
                "idxb": _wrap_idx(iB),
                "dstb": _wrap_dst(dB),
                "iota": iota_arr,
                "rec": rec_c,
            }
        )

    res = run_bass_kernel_spmd(nc, in_maps, list(range(N_CORES)))
    LAST["results"] = res
    LAST["exec_time_ns"] = res.exec_time_ns

    out_t = np.concatenate([res.results[c]["out"] for c in range(N_CORES)], axis=1)
    return np.ascontiguousarray(out_t.T[:N_NODES])


# revision 10
# speedup vs baseline: 1.3976x; 1.1393x over previous
"""GCNConv mean-aggregation kernel for 8 Trainium2 NeuronCores.

Reference computation:
    msgs   = x[src]                       # [E, D] gather
    summed = segment_sum(msgs, dst, N)    # [N, D]
    deg    = segment_sum(ones, dst, N)    # [N]
    h      = summed / max(deg, 1)
    out    = h @ W.T + b                  # [N, D_OUT]

Strategy (no collectives needed):
  - Shard edges by dst node ownership: core c owns a contiguous slice of
    6272 nodes.  Within a core, nodes are re-packed into 98 windows of 64
    via a 2D greedy balance so every window has nearly equal lo/hi edge
    counts (minimizes cross-core SPMD padding).
  - Gather tables hold x in bf16, 256B rows (64 feats + 64 zero pad);
    indices are int16, so x is split at src=32767 into lo/hi tables,
    each with a zero row at index 0 used by padding lanes.
  - Edges form two CONTINUOUS per-class streams (lo/hi), window-major,
    with no per-window 128-alignment: a 128-edge subtile may straddle two
    windows.  Each window processes its subtile range twice-shared
    boundaries included; out-of-window lanes carry dst_rel=-1 which never
    matches the iota, so they contribute zero to that window's matmul.
  - dma_gather calls cover CHUNK subtiles (single_packet coalescing; the
    SDMA engine wedges somewhere above 1024 indices per call).  Calls are
    issued as prepare_only descriptor generation + trigger_dma so the
    GPSIMD engine does not stall through the DMA drain; drains from the 4
    SWDGE queues overlap on the 16 DMA engines.
  - Aggregation per window: [128e, 64n] one-hot from dst via DVE is_equal
    (bf16), then msgs.T @ onehot accumulated into a [64, 64] PSUM tile =
    h.T directly (no transpose pass).
  - Degree reciprocals are host-computed, shipped replicated [64, NPC] in
    window-position order; normalization is one DVE multiply per window
    writing bf16 h.T slices.
  - Dense layer: z = W @ h.T per 512-column tile (bf16), bias on DVE,
    out.T written once.  Host scatters columns back to node order.
"""

import sys

sys.path.insert(0, "/opt/trn_rl_repo")

import ml_dtypes
import numpy as np

import concourse.bacc as bacc
import concourse.mybir as mybir
import concourse.tile as tile
from concourse.bass_utils import run_bass_kernel_spmd

N_NODES = 50000
N_EDGES = 800000
D = 64
N_CORES = 8
NPC = 6272          # nodes per core (= 98 windows of 64)
WIN = 64            # dst-window width per PSUM accumulation group
N_WIN = NPC // WIN  # 98
SPLIT = 32767       # src < SPLIT -> lo table, else hi table
ROW = 128           # gather row: 64 bf16 feats + 64 bf16 zero pad (256 B)
CHUNK = 8           # subtiles (of 128 edges) per dma_gather call
NQ = 4              # SWDGE queues for parallel gather descriptor work
ZCOL = 512          # output columns per dense-layer matmul tile
PREP = False        # prepare_only + trigger_dma (async drain)

F32 = mybir.dt.float32
BF16 = mybir.dt.bfloat16
I16 = mybir.dt.int16

BF = ml_dtypes.bfloat16

# Results of the most recent run (for test harness inspection).
LAST = {}


def _balance_core(a, b):
    """Greedy 2D balance of nodes (edge-count vectors a, b) into N_WIN
    bins of <=WIN slots.  Returns (win_of, slot_of) per node."""
    n = len(a)
    tA = max(a.sum() / N_WIN, 1.0)
    tB = max(b.sum() / N_WIN, 1.0)
    binA = np.zeros(N_WIN)
    binB = np.zeros(N_WIN)
    binN = np.zeros(N_WIN, dtype=np.int64)
    win_of = np.zeros(n, dtype=np.int64)
    slot_of = np.zeros(n, dtype=np.int64)
    order = np.argsort(-(a + b), kind="stable")
    for i in order:
        score = (binA + a[i]) / tA + (binB + b[i]) / tB
        score[binN >= WIN] = np.inf
        w = int(np.argmin(score))
        win_of[i] = w
        slot_of[i] = binN[w]
        binN[w] += 1
        binA[w] += a[i]
        binB[w] += b[i]
    return win_of, slot_of


def _prep(x, src, dst):
    """Host-side sharding. Returns gather tables, per-core streams and
    the shared (SPMD) program geometry."""
    x = np.asarray(x, dtype=np.float32)
    src = np.asarray(src, dtype=np.int64)
    dst = np.asarray(dst, dtype=np.int64)

    xlo = np.zeros((SPLIT + 1, ROW), dtype=BF)
    xlo[1:, :D] = x[:SPLIT].astype(BF)
    xhi = np.zeros((N_NODES - SPLIT + 1, ROW), dtype=BF)
    xhi[1:, :D] = x[SPLIT:].astype(BF)

    cls = (src >= SPLIT).astype(np.int64)
    degA = np.bincount(dst[cls == 0], minlength=N_NODES).astype(np.int64)
    degB = np.bincount(dst[cls == 1], minlength=N_NODES).astype(np.int64)
    deg = (degA + degB).astype(np.float32)
    rec = 1.0 / np.maximum(deg, 1.0)

    core_of = dst // NPC  # [E]

    # per-core window packing
    win_of = [None] * N_CORES   # node-local idx -> window
    slot_of = [None] * N_CORES
    perm = [None] * N_CORES     # position p -> window id
    cntA = np.zeros((N_CORES, N_WIN), dtype=np.int64)
    cntB = np.zeros((N_CORES, N_WIN), dtype=np.int64)
    for c in range(N_CORES):
        lo = c * NPC
        hi = min(lo + NPC, N_NODES)
        a = degA[lo:hi].astype(np.float64)
        b = degB[lo:hi].astype(np.float64)
        w_of, s_of = _balance_core(a, b)
        win_of[c], slot_of[c] = w_of, s_of
        cA = np.bincount(w_of, weights=a, minlength=N_WIN).astype(np.int64)
        cB = np.bincount(w_of, weights=b, minlength=N_WIN).astype(np.int64)
        p = np.argsort(-cA, kind="stable")
        perm[c] = p
        cntA[c] = cA[p]
        cntB[c] = cB[p]

    capA = cntA.max(axis=0)  # [N_WIN] per-position capacity, lo stream
    capB = cntB.max(axis=0)
    PA = np.zeros(N_WIN + 1, dtype=np.int64)
    np.cumsum(capA, out=PA[1:])
    PB = np.zeros(N_WIN + 1, dtype=np.int64)
    np.cumsum(capB, out=PB[1:])
    SA_sub = int(-(-PA[-1] // 128))
    SB_sub = int(-(-PB[-1] // 128))

    # view geometry (shared across cores)
    vs0A = (PA[:-1] // 128).astype(np.int64)
    vs1A = np.minimum(-(-(PA[:-1] + np.maximum(capA, 1)) // 128), SA_sub)
    vs0B = (PB[:-1] // 128).astype(np.int64)
    vs1B = np.minimum(-(-(PB[:-1] + np.maximum(capB, 1)) // 128), SB_sub)
    nvA = vs1A - vs0A
    nvB = vs1B - vs0B
    pvA = np.zeros(N_WIN + 1, dtype=np.int64)
    np.cumsum(nvA, out=pvA[1:])
    pvB = np.zeros(N_WIN + 1, dtype=np.int64)
    np.cumsum(nvB, out=pvB[1:])
    NVA = int(pvA[-1])
    NVB = int(pvB[-1])
    MAXV = int(max(nvA.max(), nvB.max()))

    geom = dict(
        capA=capA, capB=capB, PA=PA, PB=PB, SA=SA_sub, SB=SB_sub,
        vs0A=vs0A, vs1A=vs1A, vs0B=vs0B, vs1B=vs1B,
        pvA=pvA, pvB=pvB, NVA=NVA, NVB=NVB, MAXV=MAXV,
    )

    # per-core edge streams
    # group edges by (core, class, window-position)
    pos_of_win = np.zeros((N_CORES, N_WIN), dtype=np.int64)
    for c in range(N_CORES):
        pos_of_win[c][perm[c]] = np.arange(N_WIN)
    nl = dst - core_of * NPC  # node-local id
    wo = np.zeros(N_EDGES, dtype=np.int64)
    so = np.zeros(N_EDGES, dtype=np.int64)
    for c in range(N_CORES):
        m = core_of == c
        wo[m] = win_of[c][nl[m]]
        so[m] = slot_of[c][nl[m]]
    po = pos_of_win[core_of, wo]  # position of each edge's window
    key = ((core_of * 2 + cls) * N_WIN + po)
    order = np.argsort(key, kind="stable")
    src_s, so_s, key_s = src[order], so[order], key[order]
    counts = np.bincount(key_s, minlength=N_CORES * 2 * N_WIN)
    starts = np.zeros(N_CORES * 2 * N_WIN + 1, dtype=np.int64)
    np.cumsum(counts, out=starts[1:])

    per_core = []
    for c in range(N_CORES):
        iA = np.zeros(SA_sub * 128, dtype=np.int16)
        dA = np.full(SA_sub * 128, -1.0, dtype=np.float32)
        iB = np.zeros(max(SB_sub, 1) * 128, dtype=np.int16)
        dB = np.full(max(SB_sub, 1) * 128, -1.0, dtype=np.float32)
        for p in range(N_WIN):
            g = (c * 2 + 0) * N_WIN + p
            s0, s1 = starts[g], starts[g + 1]
            q0 = int(PA[p])
            iA[q0 : q0 + (s1 - s0)] = (src_s[s0:s1] + 1).astype(np.int16)
            dA[q0 : q0 + (s1 - s0)] = so_s[s0:s1].astype(np.float32)
            g = (c * 2 + 1) * N_WIN + p
            s0, s1 = starts[g], starts[g + 1]
            q0 = int(PB[p])
            iB[q0 : q0 + (s1 - s0)] = (src_s[s0:s1] - SPLIT + 1).astype(
                np.int16
            )
            dB[q0 : q0 + (s1 - s0)] = so_s[s0:s1].astype(np.float32)

        # view dst streams: mask out-of-position lanes to -1
        dvA = np.full((NVA, 128), -1.0, dtype=np.float32)
        for p in range(N_WIN):
            lo, hi = int(PA[p]), int(PA[p] + capA[p])
            for j, s in enumerate(range(int(vs0A[p]), int(vs1A[p]))):
                g0 = s * 128
                col = dvA[int(pvA[p]) + j]
                lanes = np.arange(g0, g0 + 128)
                m = (lanes >= lo) & (lanes < hi)
                col[m] = dA[lanes[m]]
        dvB = np.full((max(NVB, 1), 128), -1.0, dtype=np.float32)
        for p in range(N_WIN):
            lo, hi = int(PB[p]), int(PB[p] + capB[p])
            for j, s in enumerate(range(int(vs0B[p]), int(vs1B[p]))):
                g0 = s * 128
                col = dvB[int(pvB[p]) + j]
                lanes = np.arange(g0, g0 + 128)
                m = (lanes >= lo) & (lanes < hi)
                col[m] = dB[lanes[m]]

        # node order (position-major) for rec + output mapping
        n_nodes_c = min(NPC, N_NODES - c * NPC)
        node_pos = np.full(NPC, -1, dtype=np.int64)
        locs = pos_of_win[c][win_of[c]] * WIN + slot_of[c]
        node_pos[locs] = np.arange(n_nodes_c) + c * NPC
        rec_cols = np.ones(NPC, dtype=np.float32)
        valid = node_pos >= 0
        rec_cols[valid] = rec[node_pos[valid]]
        rec_c = np.ascontiguousarray(
            np.tile(rec_cols[None, :], (D, 1))
        )

        per_core.append(
            dict(
                iA=iA, iB=iB,
                dvA=np.ascontiguousarray(dvA.T),
                dvB=np.ascontiguousarray(dvB.T),
                rec=rec_c, node_pos=node_pos,
            )
        )

    return xlo, xhi, geom, per_core


def _wrap_idx(idx_flat):
    """int16 stream -> dma_gather layout [128, n/16]: value i at
    [i % 16, i // 16], replicated across the 8 groups of 16 partitions."""
    a = idx_flat.reshape(-1, 16).T
    return np.tile(a, (8, 1)).copy()


def _build_program(geom):
    SA, SB = geom["SA"], geom["SB"]
    NVA, NVB = geom["NVA"], geom["NVB"]
    MAXV = geom["MAXV"]
    SBp = max(SB, 1)
    NVBp = max(NVB, 1)

    nc = bacc.Bacc(
        "TRN2", target_bir_lowering=False, debug=False, num_swdge_queues=NQ
    )

    t_xlo = nc.dram_tensor("xlo", [SPLIT + 1, ROW], BF16, kind="ExternalInput")
    t_xhi = nc.dram_tensor(
        "xhi", [N_NODES - SPLIT + 1, ROW], BF16, kind="ExternalInput"
    )
    t_wt = nc.dram_tensor("wt", [D, D], BF16, kind="ExternalInput")
    t_b = nc.dram_tensor("bias", [D, 1], F32, kind="ExternalInput")
    t_ia = nc.dram_tensor("idxa", [128, SA * 8], I16, kind="ExternalInput")
    t_da = nc.dram_tensor("dsta", [128, NVA], F32, kind="ExternalInput")
    t_ib = nc.dram_tensor("idxb", [128, SBp * 8], I16, kind="ExternalInput")
    t_db = nc.dram_tensor("dstb", [128, NVBp], F32, kind="ExternalInput")
    t_iota = nc.dram_tensor("iota", [128, MAXV * WIN], F32, kind="ExternalInput")
    t_rec = nc.dram_tensor("rec", [D, NPC], F32, kind="ExternalInput")
    t_out = nc.dram_tensor("out", [D, NPC], F32, kind="ExternalOutput")

    callsA = [(p, min(CHUNK, SA - p)) for p in range(0, SA, CHUNK)]
    callsB = [(p, min(CHUNK, SB - p)) for p in range(0, SB, CHUNK)]

    with tile.TileContext(nc) as tc:
        with (
            tc.tile_pool(name="const", bufs=1) as cpool,
            tc.tile_pool(name="idx", bufs=1) as ipool,
            tc.tile_pool(name="msgsa", bufs=5) as mpa,
            tc.tile_pool(name="msgsb", bufs=4) as mpb,
            tc.tile_pool(name="oha", bufs=3) as opa,
            tc.tile_pool(name="ohb", bufs=3) as opb,
            tc.tile_pool(name="psacc", bufs=4, space="PSUM") as ps_acc,
            tc.tile_pool(name="psz", bufs=2, space="PSUM") as ps_z,
        ):
            wt_sb = cpool.tile([D, D], BF16)
            nc.sync.dma_start(out=wt_sb[:], in_=t_wt[:])
            b_sb = cpool.tile([D, 1], F32)
            nc.sync.dma_start(out=b_sb[:], in_=t_b[:])
            iota_f = cpool.tile([128, MAXV * WIN], F32)
            nc.sync.dma_start(out=iota_f[:], in_=t_iota[:])
            rec_sb = cpool.tile([D, NPC], F32)
            nc.sync.dma_start(out=rec_sb[:], in_=t_rec[:])

            ia_sb = ipool.tile([128, SA * 8], I16)
            nc.sync.dma_start(out=ia_sb[:], in_=t_ia[:])
            da_sb = ipool.tile([128, NVA], F32)
            nc.sync.dma_start(out=da_sb[:], in_=t_da[:])
            ib_sb = ipool.tile([128, SBp * 8], I16)
            nc.sync.dma_start(out=ib_sb[:], in_=t_ib[:])
            db_sb = ipool.tile([128, NVBp], F32)
            nc.sync.dma_start(out=db_sb[:], in_=t_db[:])

            ht_sb = cpool.tile([D, NPC], BF16)
            out_sb = cpool.tile([D, NPC], F32)

            chunk_tiles = {0: [], 1: []}
            call_no = [0]
            dma_sems = [
                nc.alloc_semaphore(f"gather_dma_q{q}") for q in range(NQ)
            ]

            def emit_chunk(st, k):
                if st == 0:
                    pos, nsub = callsA[k]
                    mp, tsrc, isb = mpa, t_xlo, ia_sb
                else:
                    pos, nsub = callsB[k]
                    mp, tsrc, isb = mpb, t_xhi, ib_sb
                msgs = mp.tile([128, CHUNK, ROW], BF16)
                nidx = nsub * 128
                q = call_no[0] % NQ
                nc.gpsimd.dma_gather(
                    msgs[:, :nsub, :],
                    tsrc[:],
                    isb[:, pos * 8 : pos * 8 + nsub * 8],
                    nidx,
                    nidx,
                    ROW,
                    single_packet=True,
                    prepare_only=PREP,
                    sem=dma_sems[q] if PREP else None,
                    queue_num=q,
                )
                if PREP:
                    nc.gpsimd.trigger_dma(count=None, queue_num=q)
                call_no[0] += 1
                chunk_tiles[st].append(msgs)

            cursor = [0, 0]

            def tiles_for(st, s):
                k = s // CHUNK
                while cursor[st] <= k:
                    emit_chunk(st, cursor[st])
                    cursor[st] += 1
                return chunk_tiles[st][k], s % CHUNK

            vs0A, vs1A = geom["vs0A"], geom["vs1A"]
            vs0B, vs1B = geom["vs0B"], geom["vs1B"]
            pvA, pvB = geom["pvA"], geom["pvB"]

            def onehot(op_pool, dsb, c0, nv):
                oh = op_pool.tile([128, MAXV * WIN], BF16)
                dst_b = (
                    dsb[:, c0 : c0 + nv]
                    .unsqueeze(2)
                    .to_broadcast([128, nv, WIN])
                )
                nc.vector.tensor_tensor(
                    out=oh[:, : nv * WIN].rearrange("p (s w) -> p s w", w=WIN),
                    in0=iota_f[:, : nv * WIN].rearrange(
                        "p (s w) -> p s w", w=WIN
                    ),
                    in1=dst_b,
                    op=mybir.AluOpType.is_equal,
                )
                return oh

            for p in range(N_WIN):
                nA = int(vs1A[p] - vs0A[p])
                nB = int(vs1B[p] - vs0B[p])
                ohA = onehot(opa, da_sb, int(pvA[p]), nA)
                ohB = onehot(opb, db_sb, int(pvB[p]), nB) if nB else None
                ps = ps_acc.tile([D, WIN], F32)
                tot = nA + nB
                j = 0
                for s in range(int(vs0A[p]), int(vs1A[p])):
                    msgs, col = tiles_for(0, s)
                    jj = s - int(vs0A[p])
                    nc.tensor.matmul(
                        out=ps[:],
                        lhsT=msgs[:, col, :D],
                        rhs=ohA[:, jj * WIN : (jj + 1) * WIN],
                        start=(j == 0),
                        stop=(j == tot - 1),
                    )
                    j += 1
                for s in range(int(vs0B[p]), int(vs1B[p])):
                    msgs, col = tiles_for(1, s)
                    jj = s - int(vs0B[p])
                    nc.tensor.matmul(
                        out=ps[:],
                        lhsT=msgs[:, col, :D],
                        rhs=ohB[:, jj * WIN : (jj + 1) * WIN],
                        start=(j == 0),
                        stop=(j == tot - 1),
                    )
                    j += 1
                nc.vector.tensor_tensor(
                    out=ht_sb[:, p * WIN : (p + 1) * WIN],
                    in0=ps[:],
                    in1=rec_sb[:, p * WIN : (p + 1) * WIN],
                    op=mybir.AluOpType.mult,
                )
                t0 = (p // (ZCOL // WIN)) * ZCOL
                done = (p + 1) * WIN
                if done - t0 == ZCOL or p == N_WIN - 1:
                    zc = done - t0
                    z = ps_z.tile([D, ZCOL], F32)
                    nc.tensor.matmul(
                        out=z[:, :zc],
                        lhsT=wt_sb[:],
                        rhs=ht_sb[:, t0 : t0 + zc],
                        start=True,
                        stop=True,
                    )
                    nc.vector.tensor_scalar_add(
                        out_sb[:, t0 : t0 + zc], z[:, :zc], b_sb[:]
                    )

            nc.sync.dma_start(out=t_out[:], in_=out_sb[:])

    nc.compile()
    return nc


def kernel(x, src, dst, W, b):
    x = np.asarray(x, dtype=np.float32)
    W = np.asarray(W, dtype=np.float32)
    b = np.asarray(b, dtype=np.float32)

    xlo, xhi, geom, per_core = _prep(x, src, dst)
    nc = _build_program(geom)

    wt = np.ascontiguousarray(W.T).astype(BF)
    bcol = np.ascontiguousarray(b.reshape(D, 1))
    iota_arr = np.tile(
        np.arange(WIN, dtype=np.float32)[None, :], (128, geom["MAXV"])
    ).copy()

    in_maps = []
    for c in range(N_CORES):
        pc = per_core[c]
        in_maps.append(
            {
                "xlo": xlo,
                "xhi": xhi,
                "wt": wt,
                "bias": bcol,
                "idxa": _wrap_idx(pc["iA"]),
                "dsta": pc["dvA"],
                "idxb": _wrap_idx(pc["iB"]),
                "dstb": pc["dvB"],
                "iota": iota_arr,
                "rec": pc["rec"],
            }
        )

    res = run_bass_kernel_spmd(nc, in_maps, list(range(N_CORES)))
    LAST["results"] = res
    LAST["exec_time_ns"] = res.exec_time_ns

    out = np.zeros((N_NODES, D), dtype=np.float32)
    for c in range(N_CORES):
        cols = res.results[c]["out"]  # [D, NPC]
        node_pos = per_core[c]["node_pos"]
        valid = node_pos >= 0
        out[node_pos[valid]] = cols[:, valid].T
    return np.ascontiguousarray(out)
